# revision 6
# baseline (speedup 1.0000x reference)
"""Trainium2 Bass kernel for nn_AddDropMRR (add-drop microring resonator).

Math: rotate the complex plane per wavelength by -arg(G) (magnitudes are
invariant), where G = t2*s1/den is the resonance response. Then with
u = P*x + s*a (the ring field after coupler 1 + add), the two outputs
collapse to two-term sums of squares with REAL per-wavelength coefficients:

  through^2 = (g*v)^2 + (c2*x)^2,   v = r*x + u,  g = |G|,  r = t1*den_re/(t2*s1*g)...
              (r = (t1/(t2*s1))*den_re, g*r = t1*cos(argG))
  drop^2    = k2c^2*u^2 + (k2c*Q*x)^2

All coefficients (P, r, g, c2^2, (k2c*Q)^2) depend only on `wavelengths`
(8192 values) and the scalar params, so they are computed on HOST and DMA'd
as a tiny [128, 5*nchunk] f32 table. The device graph is pure streaming:
no trig, no table switches (only Square/Sqrt, which share one ACT table).

Engine split per [128, 2048] chunk (cost-model ns):
  Pool (gpsimd): u, v, D2 accumulation  — 3 STT x 1707
  DVE (vector):  x^2, u^2 (TT 2x fp16), two TS-ptr (4x fp16), W2 add — 4569
  ACT (scalar):  (g/32*v)^2 fused square, two Sqrt(1024*.) — 5675
  DMA: 4 x 512KB fp16 — 6316  <- roofline

Tensors ride fp16 (2-byte enables DVE 2x/4x perf modes; 11-bit mantissa
beats bf16 precision); the /32 and *1024 rescales keep (g*v)^2 < 65504.
Sharding: wavelength dim split 8 ways, host-transposed so wavelength lies
on SBUF partitions; per-wavelength coefficients are [128,1] f32 scalars.
"""
import numpy as np

B = 2048           # batch
W = 8192           # wavelengths
NCORES = 8
WSH = W // NCORES  # 1024 wavelengths per core
P = 128            # SBUF partitions
NCHUNK = WSH // P  # 8 chunks per core
NCOEF = 5          # P, r, g/32, (c2/32)^2, (8*k2c*Q)^2
N_EFF = 2.4
CIRC = 2.0 * np.pi * 1e-05


def _host_prep(wavelengths, coupling_1, coupling_2, phi_1, phi_2, phi_ring,
               alpha):
    """Scalars + per-wavelength coefficient table [NCOEF, W] f32."""
    c1 = float(np.asarray(coupling_1).reshape(-1)[0])
    c2 = float(np.asarray(coupling_2).reshape(-1)[0])
    p1 = float(np.asarray(phi_1).reshape(-1)[0])
    pr = float(np.asarray(phi_ring).reshape(-1)[0])
    al = float(np.asarray(alpha).reshape(-1)[0])
    k1c = float(np.clip(c1, 0.01, 0.99))
    k2c = float(np.clip(c2, 0.01, 0.99))
    t1 = float(np.sqrt(1.0 - k1c * k1c))
    t2 = float(np.sqrt(1.0 - k2c * k2c))
    s = float(np.sqrt(c2))       # unclamped, as in reference
    s1 = float(np.sqrt(c1))      # unclamped
    kappa = float(al * np.sqrt(1.0 - c1 * c1) * np.sqrt(1.0 - c2 * c2))

    # phi in f32 exactly as the reference computes it, then f64 trig
    wl = np.asarray(wavelengths, np.float32)
    phi32 = (np.float32(2.0 * np.pi * N_EFF) / wl) * np.float32(CIRC) \
        + np.float32(pr)
    phi = phi32.astype(np.float64)
    sin_p = np.sin(phi + p1)
    cos_p = np.cos(phi + p1)
    sin_f = np.sin(phi)
    cos_f = np.cos(phi)

    Pv = -k1c * al * sin_p
    Qv = k1c * al * cos_p
    den_re = 1.0 - kappa * cos_f
    den2 = den_re * den_re + (kappa * sin_f) ** 2
    rsq = 1.0 / np.sqrt(den2)
    g = (t2 * s1) * rsq
    r = (t1 / (t2 * s1)) * den_re
    c2v = (t2 * s1 * Qv - t1 * kappa * sin_f) * rsq

    coefs = np.stack([
        Pv,
        r + Pv,          # v = (r+P)*x + a' directly (v = r*x + u)
        g / 32.0,
        (c2v / 32.0) ** 2,
        Qv ** 2,
    ]).astype(np.float32)                       # [NCOEF, W]
    return coefs, dict(s=s, k2c=k2c, rp=(r + Pv).astype(np.float32))


def _build_graph(k2c, loop_n=1, nchunk=NCHUNK, bufs=6,
                 uv="tstt", d2eng="gp", uueng="dve"):
    """SPMD per-core graph.
    uv:   'tstt' = linear forms via TS-ptr (4x fp16) + TT add (2x fp16);
          'stt'  = single fused scalar_tensor_tensor (no DVE perf mode).
    d2eng: engine for the D2 = uu + t2 add ('gp' gpsimd | 'dve').
    uueng: engine for u^2 ('dve' TT | 'act' Square).
    loop_n>1 wraps the body in an on-device For_i loop for timing."""
    import concourse.tile as tile
    from concourse import bacc, mybir

    f32 = mybir.dt.float32
    f16 = mybir.dt.float16
    AF = mybir.ActivationFunctionType
    ALU = mybir.AluOpType

    wsh = nchunk * P
    nc = bacc.Bacc("TRN2", target_bir_lowering=False, debug=False,
                   num_devices=NCORES)
    x_ext = nc.declare_dram_parameter("x_t", [wsh, B], f16, isOutput=False)
    a_ext = nc.declare_dram_parameter("a_t", [wsh, B], f16, isOutput=False)
    cf_ext = nc.declare_dram_parameter("cf_t", [P, NCOEF * nchunk], f32,
                                       isOutput=False)
    o1_ext = nc.declare_dram_parameter("o1_t", [wsh, B], f16, isOutput=True)
    o2_ext = nc.declare_dram_parameter("o2_t", [wsh, B], f16, isOutput=True)

    k2sq = float(k2c * k2c)

    with tile.TileContext(nc) as tc:
        with tc.tile_pool(name="cst", bufs=1) as cst, \
             tc.tile_pool(name="mio", bufs=bufs) as mio:

            def body(_iv=None):
                cf = cst.tile([P, NCOEF * nchunk], f32, tag="cf", name="cf")
                nc.sync.dma_start(cf[:], cf_ext[:])

                def C(k, c):
                    i = k * nchunk + c
                    return cf[:, i:i + 1]

                for c in range(nchunk):
                    rs = slice(c * P, (c + 1) * P)
                    xt = mio.tile([P, B], f16, tag="xt", name="xt")
                    nc.sync.dma_start(xt[:], x_ext[rs, :])
                    at = mio.tile([P, B], f16, tag="at", name="at")
                    nc.sync.dma_start(at[:], a_ext[rs, :])
                    vt = mio.tile([P, B], f16, tag="vt", name="vt")
                    ut = mio.tile([P, B], f16, tag="ut", name="ut")
                    if uv == "stt":
                        # u = P*x + a' ; v = (r+P)*x + a'
                        nc.vector.scalar_tensor_tensor(
                            out=ut[:], in0=xt[:], scalar=C(0, c), in1=at[:],
                            op0=ALU.mult, op1=ALU.add)
                        nc.vector.scalar_tensor_tensor(
                            out=vt[:], in0=xt[:], scalar=C(1, c), in1=at[:],
                            op0=ALU.mult, op1=ALU.add)
                    else:
                        # TS-ptr (4x) then TT add (2x)
                        nc.vector.tensor_scalar(ut[:], xt[:], C(0, c), None,
                                                ALU.mult)
                        nc.vector.tensor_add(ut[:], ut[:], at[:])
                        nc.vector.tensor_scalar(vt[:], xt[:], C(1, c), None,
                                                ALU.mult)
                        nc.vector.tensor_add(vt[:], vt[:], at[:])
                    # vv = (g/32 * v)^2   (ACT, in place)
                    nc.scalar.activation(vt[:], vt[:], AF.Square,
                                         scale=C(2, c))
                    # xx = x^2 (in place), uu = u^2
                    nc.vector.tensor_mul(xt[:], xt[:], xt[:])
                    if uueng == "act":
                        nc.scalar.activation(ut[:], ut[:], AF.Square)
                    else:
                        nc.vector.tensor_mul(ut[:], ut[:], ut[:])
                    # t1 = (c2/32)^2*xx (into a'); t2 = Q^2*xx (in place)
                    nc.vector.tensor_scalar(at[:], xt[:], C(3, c), None,
                                            ALU.mult)
                    nc.vector.tensor_scalar(xt[:], xt[:], C(4, c), None,
                                            ALU.mult)
                    # W2 = vv + t1 ; D2 = uu + t2
                    nc.vector.tensor_add(vt[:], vt[:], at[:])
                    if d2eng == "gp":
                        nc.gpsimd.tensor_add(ut[:], ut[:], xt[:])
                    else:
                        nc.vector.tensor_add(ut[:], ut[:], xt[:])
                    # through = sqrt(1024*W2'), drop = sqrt(k2c^2*D2)
                    nc.scalar.activation(vt[:], vt[:], AF.Sqrt, scale=1024.0)
                    nc.scalar.activation(ut[:], ut[:], AF.Sqrt, scale=k2sq)
                    nc.sync.dma_start(o1_ext[rs, :], vt[:])
                    nc.sync.dma_start(o2_ext[rs, :], ut[:])

            if loop_n > 1:
                with tc.For_i(0, loop_n, 1):
                    body()
            else:
                body()

    nc.compile()
    return nc


def _shard_inputs(input_signal, add_signal, coefs, s):
    x = np.asarray(input_signal, dtype=np.float32).astype(np.float16)
    a = (np.asarray(add_signal, dtype=np.float32)
         * np.float32(s)).astype(np.float16)
    in_maps = []
    for i in range(NCORES):
        sl = slice(i * WSH, (i + 1) * WSH)
        # coef layout [P, NCOEF*NCHUNK]: column k*NCHUNK+c holds coef k of
        # chunk c; row p is wavelength c*P+p within the shard.
        cf = np.ascontiguousarray(
            coefs[:, sl].reshape(NCOEF, NCHUNK, P)
            .transpose(2, 0, 1).reshape(P, NCOEF * NCHUNK))
        in_maps.append({
            "x_t": np.ascontiguousarray(x[:, sl].T),
            "a_t": np.ascontiguousarray(a[:, sl].T),
            "cf_t": cf,
        })
    return in_maps


def _gather_outputs(results):
    through = np.empty((B, W), np.float32)
    drop = np.empty((B, W), np.float32)
    for i in range(NCORES):
        sl = slice(i * WSH, (i + 1) * WSH)
        through[:, sl] = results[i]["o1_t"].T.astype(np.float32)
        drop[:, sl] = results[i]["o2_t"].T.astype(np.float32)
    return through, drop


def kernel(input_signal, add_signal, wavelengths, coupling_1, coupling_2,
           phi_1, phi_2, phi_ring, alpha):
    from concourse.bass_utils import run_bass_kernel_spmd

    coefs, sc = _host_prep(wavelengths, coupling_1, coupling_2, phi_1, phi_2,
                           phi_ring, alpha)
    nc = _build_graph(sc["k2c"])
    in_maps = _shard_inputs(input_signal, add_signal, coefs, sc["s"])
    res = run_bass_kernel_spmd(nc, in_maps, core_ids=list(range(NCORES)))
    return _gather_outputs(res.results)


# revision 7
# speedup vs baseline: 1.0462x; 1.0462x over previous
"""Trainium2 Bass kernel for nn_AddDropMRR (add-drop microring resonator).

Math: rotate the complex plane per wavelength by -arg(G) (magnitudes are
invariant), where G = t2*s1/den is the ring response. With u = P*x + s*a:

  through^2 = (g*v)^2 + (c2*x)^2,   v = (r+P)*x + s*a,  g = |G|
  drop^2    = k2c^2*(u^2 + Q^2*x^2)

All per-wavelength coefficients depend only on `wavelengths` (8192 values)
and scalar params -> computed on HOST, DMA'd as tiny f32 tables. The device
graph is pure streaming with work spread over ALL FIVE engine queues
(measured per-[128,2048]-chunk costs):

  PE:    v = diag(r+P) @ x  (+)  I @ a'   (8 matmuls into PSUM, ~3.4us)
  ACT:   vv = (g/32 * v_psum)^2, 2x Sqrt, issues the 2 output DMAs (~7.4us)
  DVE:   u (TS-ptr 4x + TT 2x fp16), x^2, t1, t2, W2, D2 adds (~7.4us)
  GP:    u^2 (Pool/gpsimd tensor_mul, ~5.6us)
  DMA:   in on qSP ring, out on qAct ring (~6.9us saturated @ ~290GB/s)

Tensors ride fp16 (2-byte dtype enables DVE 2x/4x perf modes; better
mantissa than bf16); the /32, *1024 rescales keep (g*v)^2 under fp16 max.
Sharding: wavelength dim split 8 ways across cores, host-transposed so
wavelength lies on SBUF partitions; coefficients are [128,1] f32 scalars.
"""
import numpy as np

B = 2048           # batch
W = 8192           # wavelengths
NCORES = 8
WSH = W // NCORES  # 1024 wavelengths per core
P = 128            # SBUF partitions
NCHUNK = WSH // P  # 8 chunks per core
NCOEF = 4          # P, g/32, (c2/32)^2, Q^2
N_EFF = 2.4
CIRC = 2.0 * np.pi * 1e-05


def _host_prep(wavelengths, coupling_1, coupling_2, phi_1, phi_2, phi_ring,
               alpha):
    """Scalars, coefficient table [NCOEF, W] f32, diag blocks [W] (r+P)."""
    c1 = float(np.asarray(coupling_1).reshape(-1)[0])
    c2 = float(np.asarray(coupling_2).reshape(-1)[0])
    p1 = float(np.asarray(phi_1).reshape(-1)[0])
    pr = float(np.asarray(phi_ring).reshape(-1)[0])
    al = float(np.asarray(alpha).reshape(-1)[0])
    k1c = float(np.clip(c1, 0.01, 0.99))
    k2c = float(np.clip(c2, 0.01, 0.99))
    t1 = float(np.sqrt(1.0 - k1c * k1c))
    t2 = float(np.sqrt(1.0 - k2c * k2c))
    s = float(np.sqrt(c2))       # unclamped, as in reference
    s1 = float(np.sqrt(c1))      # unclamped
    kappa = float(al * np.sqrt(1.0 - c1 * c1) * np.sqrt(1.0 - c2 * c2))

    # phi in f32 exactly as the reference computes it, then f64 trig
    wl = np.asarray(wavelengths, np.float32)
    phi32 = (np.float32(2.0 * np.pi * N_EFF) / wl) * np.float32(CIRC) \
        + np.float32(pr)
    phi = phi32.astype(np.float64)
    sin_p = np.sin(phi + p1)
    cos_p = np.cos(phi + p1)
    sin_f = np.sin(phi)
    cos_f = np.cos(phi)

    Pv = -k1c * al * sin_p
    Qv = k1c * al * cos_p
    den_re = 1.0 - kappa * cos_f
    den2 = den_re * den_re + (kappa * sin_f) ** 2
    rsq = 1.0 / np.sqrt(den2)
    g = (t2 * s1) * rsq
    r = (t1 / (t2 * s1)) * den_re
    c2v = (t2 * s1 * Qv - t1 * kappa * sin_f) * rsq

    coefs = np.stack([
        Pv,
        g / 32.0,
        (c2v / 32.0) ** 2,
        Qv ** 2,
    ]).astype(np.float32)                       # [NCOEF, W]
    return coefs, dict(s=s, k2c=k2c, rp=(r + Pv).astype(np.float32))


def _build_graph(k2c, loop_n=1, nchunk=NCHUNK, bufs=6, pe=True, uueng="gp",
                 split_dma=True, vv_split=1):
    """SPMD per-core graph.
    pe: v via PE diag-matmul into PSUM (else DVE TS+TT like u).
    uueng: engine for u^2 ('gp' | 'dve').
    split_dma: stores issued from the ACT HWDGE ring instead of qSP.
    vv_split: number of ACT ops covering the vv square (PSUM read), 1 or 4.
    loop_n>1 wraps the body in an on-device For_i loop for timing."""
    import concourse.tile as tile
    from concourse import bacc, mybir, bass

    f32 = mybir.dt.float32
    f16 = mybir.dt.float16
    AF = mybir.ActivationFunctionType
    ALU = mybir.AluOpType

    wsh = nchunk * P
    nc = bacc.Bacc("TRN2", target_bir_lowering=False, debug=False,
                   num_devices=NCORES)
    x_ext = nc.declare_dram_parameter("x_t", [wsh, B], f16, isOutput=False)
    a_ext = nc.declare_dram_parameter("a_t", [wsh, B], f16, isOutput=False)
    cf_ext = nc.declare_dram_parameter("cf_t", [P, NCOEF * nchunk], f32,
                                       isOutput=False)
    dg_ext = nc.declare_dram_parameter("dg_t", [P, (nchunk + 1) * P], f16,
                                       isOutput=False)
    o1_ext = nc.declare_dram_parameter("o1_t", [wsh, B], f16, isOutput=True)
    o2_ext = nc.declare_dram_parameter("o2_t", [wsh, B], f16, isOutput=True)

    k2sq = float(k2c * k2c)
    out_eng = "scalar" if split_dma else "sync"

    with tile.TileContext(nc) as tc:
        with tc.tile_pool(name="cst", bufs=1) as cst, \
             tc.tile_pool(name="mio", bufs=bufs) as mio, \
             tc.tile_pool(name="psum", bufs=2,
                          space=bass.MemorySpace.PSUM) as psum:

            def body(_iv=None):
                cf = cst.tile([P, NCOEF * nchunk], f32, tag="cf", name="cf")
                nc.sync.dma_start(cf[:], cf_ext[:])
                dg = cst.tile([P, (nchunk + 1) * P], f16, tag="dg", name="dg")
                if pe:
                    nc.sync.dma_start(dg[:], dg_ext[:])

                def C(k, c):
                    return cf[:, k * nchunk + c:k * nchunk + c + 1]

                for c in range(nchunk):
                    rs = slice(c * P, (c + 1) * P)
                    xt = mio.tile([P, B], f16, tag="xt", name="xt")
                    nc.sync.dma_start(xt[:], x_ext[rs, :])
                    at = mio.tile([P, B], f16, tag="at", name="at")
                    nc.sync.dma_start(at[:], a_ext[rs, :])
                    vt = mio.tile([P, B], f16, tag="vt", name="vt")
                    # ---- v = (r+P)*x + a' ----
                    if pe:
                        vps = psum.tile([P, B], f32, tag="vps", name="vps")
                        dgc = dg[:, c * P:(c + 1) * P]
                        ide = dg[:, nchunk * P:(nchunk + 1) * P]
                        for j in range(0, B, 512):
                            nc.tensor.matmul(vps[:, j:j + 512], dgc,
                                             xt[:, j:j + 512],
                                             start=True, stop=False)
                        for j in range(0, B, 512):
                            nc.tensor.matmul(vps[:, j:j + 512], ide,
                                             at[:, j:j + 512],
                                             start=False, stop=True)
                    # ---- u = P*x + a'  (DVE TS 4x + TT 2x) ----
                    ut = mio.tile([P, B], f16, tag="ut", name="ut")
                    nc.vector.tensor_scalar(ut[:], xt[:], C(0, c), None,
                                            ALU.mult)
                    nc.vector.tensor_add(ut[:], ut[:], at[:])
                    if not pe:
                        # fallback: v on DVE too (needs rp in a coef slot;
                        # kept only for A/B probing - uses C(1) scaled wrong)
                        nc.vector.tensor_scalar(vt[:], xt[:], C(1, c), None,
                                                ALU.mult)
                        nc.vector.tensor_add(vt[:], vt[:], at[:])
                    # ---- uu = u^2 ----
                    if uueng == "gp":
                        nc.gpsimd.tensor_mul(ut[:], ut[:], ut[:])
                    else:
                        nc.vector.tensor_mul(ut[:], ut[:], ut[:])
                    # ---- vv = (g/32 * v)^2 (ACT, from PSUM if pe) ----
                    if pe:
                        if vv_split == 1:
                            nc.scalar.activation(vt[:], vps[:], AF.Square,
                                                 scale=C(1, c))
                        else:
                            step = B // vv_split
                            for j in range(0, B, step):
                                nc.scalar.activation(
                                    vt[:, j:j + step], vps[:, j:j + step],
                                    AF.Square, scale=C(1, c))
                    else:
                        nc.scalar.activation(vt[:], vt[:], AF.Square,
                                             scale=C(1, c))
                    # ---- xx = x^2 (in place) ----
                    nc.vector.tensor_mul(xt[:], xt[:], xt[:])
                    # ---- t1 = (c2/32)^2*xx (into a'); W2 = vv + t1 ----
                    nc.vector.tensor_scalar(at[:], xt[:], C(2, c), None,
                                            ALU.mult)
                    nc.vector.tensor_add(vt[:], vt[:], at[:])
                    # ---- t2 = Q^2*xx (in place); D2 = uu + t2 ----
                    nc.vector.tensor_scalar(xt[:], xt[:], C(3, c), None,
                                            ALU.mult)
                    nc.vector.tensor_add(ut[:], ut[:], xt[:])
                    # ---- through = sqrt(1024*W2'), drop = sqrt(k2c^2*D2) ----
                    nc.scalar.activation(vt[:], vt[:], AF.Sqrt, scale=1024.0)
                    nc.scalar.activation(ut[:], ut[:], AF.Sqrt, scale=k2sq)
                    getattr(nc, out_eng).dma_start(o1_ext[rs, :], vt[:])
                    getattr(nc, out_eng).dma_start(o2_ext[rs, :], ut[:])

            if loop_n > 1:
                with tc.For_i(0, loop_n, 1):
                    body()
            else:
                body()

    nc.compile()
    return nc


def _shard_inputs(input_signal, add_signal, coefs, s, rp=None):
    x = np.asarray(input_signal, dtype=np.float32).astype(np.float16)
    a = (np.asarray(add_signal, dtype=np.float32)
         * np.float32(s)).astype(np.float16)
    in_maps = []
    for i in range(NCORES):
        sl = slice(i * WSH, (i + 1) * WSH)
        # coef layout [P, NCOEF*NCHUNK]: column k*NCHUNK+c holds coef k of
        # chunk c; row p is wavelength c*P+p within the shard.
        cf = np.ascontiguousarray(
            coefs[:, sl].reshape(NCOEF, NCHUNK, P)
            .transpose(2, 0, 1).reshape(P, NCOEF * NCHUNK))
        m = {
            "x_t": np.ascontiguousarray(x[:, sl].T),
            "a_t": np.ascontiguousarray(a[:, sl].T),
            "cf_t": cf,
        }
        # diag blocks: dg[:, c*P:(c+1)*P] = diag(rp[shard, chunk c]);
        # last block = identity.
        dgm = np.zeros((P, (NCHUNK + 1) * P), np.float16)
        if rp is not None:
            rsh = rp[sl].reshape(NCHUNK, P)
            for c in range(NCHUNK):
                dgm[:, c * P:(c + 1) * P] = np.diag(rsh[c].astype(np.float16))
        dgm[:, NCHUNK * P:] = np.eye(P, dtype=np.float16)
        m["dg_t"] = dgm
        in_maps.append(m)
    return in_maps


def _gather_outputs(results):
    through = np.empty((B, W), np.float32)
    drop = np.empty((B, W), np.float32)
    for i in range(NCORES):
        sl = slice(i * WSH, (i + 1) * WSH)
        through[:, sl] = results[i]["o1_t"].T.astype(np.float32)
        drop[:, sl] = results[i]["o2_t"].T.astype(np.float32)
    return through, drop


def kernel(input_signal, add_signal, wavelengths, coupling_1, coupling_2,
           phi_1, phi_2, phi_ring, alpha):
    from concourse.bass_utils import run_bass_kernel_spmd

    coefs, sc = _host_prep(wavelengths, coupling_1, coupling_2, phi_1, phi_2,
                           phi_ring, alpha)
    nc = _build_graph(sc["k2c"])
    in_maps = _shard_inputs(input_signal, add_signal, coefs, sc["s"],
                            rp=sc["rp"])
    res = run_bass_kernel_spmd(nc, in_maps, core_ids=list(range(NCORES)))
    return _gather_outputs(res.results)
